# revision 14
# baseline (speedup 1.0000x reference)
"""Causal self-attention (k/q swapped variant) on 8 Trainium2 NeuronCores.

Problem (hardcoded shapes): B=2, N=2048, D=1024, H=16, DH=64.
  kqv = einsum('bnd,hde->bhne', x, Wkqv) + bkqv   ; split -> k, q, v
  A[b,h,n,m] = k[b,h,n]·q[b,h,m] / sqrt(DH), causal mask m<=n, softmax over m
  sa = A @ v ; concat heads ; out = sa @ Wo + bo

Sharding: tensor-parallel over heads — each core owns 2 heads (both batch
entries), computes its partial output projection sa_local @ Wo[rows], and the
host sums the 8 partials (+bo).

Per-core device kernel (all matmul operands bf16, fp32 PSUM accumulation).
v3 schedule, built from trace analysis of v1 (211.8us) and v2 (184.6us):
  - The PE activity monitor (HAM) halves the PE clock whenever the PE is
    not ~100% busy across a 3.4us window, so the whole program is one
    gap-free PE stream: blocks [b0j0 b0j1 b0j2 b1j0 b0j3 b1j3 b1j2 b1j1]
    with every non-attention PE unit (projection groups, v-transposes,
    output-projection blocks) woven between attention chunks so each
    phase has more PE work than ACT(exp) work. The stream ends on the
    smallest block so the un-weavable tail is short.
  - scores for the two heads are emitted back-to-back as K=64 matmuls into
    partitions 0:64 / 64:128 (row-tiled halves of the PE array) -> they
    execute concurrently (verified dstart ~3ns, ~2x on scores).
  - exp is one ACTIVATE per chunk over both heads via a strided [128,2,W]
    view of a 2-bank [128,1024] PSUM tile; PV matmuls are emitted one slot
    late so the strict-FIFO PE never head-of-line blocks on ACT.
  - DMA trigger instructions live ONLY on the sync queue: in v2 the xt
    triggers on the scalar queue waited 10-35us for DMA-ring space and
    head-of-line blocked the first exps (-> 13.7us HAM window). Deep
    ob buffering (8) absorbs the out-DMA triggers queueing behind the
    still-waiting input triggers.
  - engine budget: ACT = exp (~85us at the measured ~1.0ns/elem) + b0
    denominator copies + mid-game outproj copies; DVE = biases + softmax
    normalize + endgame outproj copies; GpSimd = causal-mask multiplies
    only; PE ~135us = the global pacer.
  - output staged and DMAed as bf16 (halves out traffic; host sums the
    8 bf16 partials in f32, ~0.3% added noise vs the 2% gate).
"""

import numpy as np
import ml_dtypes

B = 2
N = 2048
D = 1024
H = 16
DH = 64
NCORES = 8
HL = H // NCORES          # heads per core = 2
DC = D // 128             # contraction chunks = 8
NB = N // 128             # 128-row blocks = 16
NJ = N // 512             # 512-col blocks = 4
NWARM = 72                # PE warmup matmuls during DMA prologue

BF16 = ml_dtypes.bfloat16

_CACHE = {}


def _build():
    import concourse.bass as bass
    import concourse.mybir as mybir
    import concourse.tile as tile
    from concourse import bacc
    from contextlib import ExitStack

    f32 = mybir.dt.float32
    bf16 = mybir.dt.bfloat16
    Exp = mybir.ActivationFunctionType.Exp

    nc = bacc.Bacc("TRN2", target_bir_lowering=False, debug=False,
                   enable_asserts=False, num_devices=NCORES)

    xt_d = nc.dram_tensor("xt", [B, D, N], bf16, kind="ExternalInput")
    # k/q/v weights arrive pre-shuffled to the SBUF layout [128, DC*128]
    # (partition = within-chunk row, free = (chunk, head-col))
    wk_d = nc.dram_tensor("wk2", [128, DC * 128], bf16, kind="ExternalInput")
    wq_d = nc.dram_tensor("wq2", [128, DC * 128], bf16, kind="ExternalInput")
    wv_d = nc.dram_tensor("wv2", [128, DC * 128], bf16, kind="ExternalInput")
    wo_d = nc.dram_tensor("wo2", [128, D], bf16, kind="ExternalInput")
    bk_d = nc.dram_tensor("bk2", [128, 1], f32, kind="ExternalInput")
    bq_d = nc.dram_tensor("bq2", [128, 1], f32, kind="ExternalInput")
    bv_d = nc.dram_tensor("bv2", [128, 1], f32, kind="ExternalInput")
    eye_d = nc.dram_tensor("eye128", [128, 128], bf16, kind="ExternalInput")
    m01_d = nc.dram_tensor("m01", [128, 128], bf16, kind="ExternalInput")
    out_d = nc.dram_tensor("out", [B, N, D], bf16, kind="ExternalOutput")

    with tile.TileContext(nc) as tc, ExitStack() as ctx:
        const = ctx.enter_context(tc.tile_pool(name="const", bufs=1))
        xt_pool = ctx.enter_context(tc.tile_pool(name="xt", bufs=1))
        kq_pool = ctx.enter_context(tc.tile_pool(name="kq", bufs=6))
        v_pool = ctx.enter_context(tc.tile_pool(name="v", bufs=2))
        sa_pool = ctx.enter_context(tc.tile_pool(name="sa", bufs=2))
        pt_pool = ctx.enter_context(tc.tile_pool(name="pt", bufs=4))
        rc_pool = ctx.enter_context(tc.tile_pool(name="rc", bufs=2))
        ob_pool = ctx.enter_context(tc.tile_pool(name="ob", bufs=8))
        # PSUM: s 2x[128,1024] = 4 banks, pv 2x[128,512] = 2, po 2x = 2.
        s_ps = ctx.enter_context(tc.tile_pool(name="s_ps", bufs=2, space="PSUM"))
        pv_ps = ctx.enter_context(tc.tile_pool(name="pv_ps", bufs=2, space="PSUM"))
        po_ps = ctx.enter_context(tc.tile_pool(name="po_ps", bufs=2, space="PSUM"))

        # ---- SBUF consts / inputs
        m01_sb = const.tile([128, 128], bf16, name="m01_sb")
        eye_sb = const.tile([128, 128], bf16, name="eye_sb")
        wk_sb = const.tile([128, DC * 128], bf16, name="wk_sb")
        wq_sb = const.tile([128, DC * 128], bf16, name="wq_sb")
        wv_sb = const.tile([128, DC * 128], bf16, name="wv_sb")
        wo_sb = const.tile([128, D], bf16, name="wo_sb")
        bk_sb = const.tile([128, 1], f32, name="bk_sb")
        bq_sb = const.tile([128, 1], f32, name="bq_sb")
        bv_sb = const.tile([128, 1], f32, name="bv_sb")

        xt0 = {}   # (dc, half) -> [128, 1024] piece of batch 0
        xt1 = {}   # dc -> [128, 2048] of batch 1

        k2 = {}
        q2 = {}
        vt = {}
        v_sb = {}
        sa_sb = {}
        for b in range(B):
            k2[b] = kq_pool.tile([128, N], bf16, name=f"k2_b{b}", tag="kq")
            q2[b] = kq_pool.tile([128, N], bf16, name=f"q2_b{b}", tag="kq")
            vt[b] = kq_pool.tile([128, N], bf16, name=f"vt_b{b}", tag="kq")
            v_sb[b] = v_pool.tile([128, NB * 192], bf16, name=f"v_b{b}", tag="v")
            sa_sb[b] = sa_pool.tile([128, N], bf16, name=f"sa_b{b}", tag="sa")

        # ones columns of v_sb (denominator trick) — DVE, no input deps
        for b in range(B):
            nc.vector.memset(
                v_sb[b][:].rearrange("p (nb g) -> p nb g", g=192)[:, :, 64:128],
                1.0)

        # ---- DMA issue order, ALL on the sync queue (a trigger on a
        # compute queue head-of-line blocks that engine while it waits for
        # DMA-ring space): warmup const -> weights -> xt0 half0 pieces ->
        # small consts -> xt0 half1 -> xt1.
        nc.sync.dma_start(m01_sb[:], m01_d.ap())
        nc.sync.dma_start(wk_sb[:], wk_d.ap())
        nc.sync.dma_start(wq_sb[:], wq_d.ap())
        nc.sync.dma_start(wv_sb[:], wv_d.ap())
        for dc in range(DC):
            t = xt_pool.tile([128, 1024], bf16, name=f"xt0_{dc}_0",
                             tag="xt0", bufs=2 * DC)
            nc.sync.dma_start(t[:], xt_d.ap()[0, dc * 128:(dc + 1) * 128, 0:1024])
            xt0[dc, 0] = t
        nc.sync.dma_start(bk_sb[:], bk_d.ap())
        nc.sync.dma_start(bq_sb[:], bq_d.ap())
        nc.sync.dma_start(bv_sb[:], bv_d.ap())
        nc.sync.dma_start(eye_sb[:], eye_d.ap())
        nc.sync.dma_start(wo_sb[:], wo_d.ap())
        for dc in range(DC):
            t = xt_pool.tile([128, 1024], bf16, name=f"xt0_{dc}_1",
                             tag="xt0", bufs=2 * DC)
            nc.sync.dma_start(t[:], xt_d.ap()[0, dc * 128:(dc + 1) * 128, 1024:2048])
            xt0[dc, 1] = t
        for dc in range(DC):
            t = xt_pool.tile([128, N], bf16, name=f"xt1_{dc}", tag="xt1",
                             bufs=DC)
            nc.sync.dma_start(t[:], xt_d.ap()[1, dc * 128:(dc + 1) * 128, :])
            xt1[dc] = t

        def xt_ap(b, dc, c0, c1):
            if b == 1:
                return xt1[dc][:, c0:c1]
            half = c0 // 1024
            return xt0[dc, half][:, c0 - half * 1024:c1 - half * 1024]

        # ---- PE warmup: dense dummy matmuls on the first-arriving const so
        # HAM un-throttles (k=8/8) before the real stream begins.
        wu = po_ps.tile([128, 512], f32, name="wu", tag="po")
        for _ in range(NWARM):
            nc.tensor.matmul(wu[:, 0:128], m01_sb[:], m01_sb[:],
                             start=True, stop=True)

        gdef = {"k": (wk_sb, bk_sb, k2), "q": (wq_sb, bq_sb, q2),
                "v": (wv_sb, bv_sb, vt)}

        def proj_group(b, g, nj):
            """One [128, 512] projection psum group -> dst cols of k/q/vT."""
            w_sb, bias_sb, dstd = gdef[g]
            ps = po_ps.tile([128, 512], f32, name="proj_ps", tag="po")
            for dc in range(DC):
                nc.tensor.matmul(
                    ps[:], w_sb[:, dc * 128:(dc + 1) * 128],
                    xt_ap(b, dc, nj * 512, (nj + 1) * 512),
                    start=(dc == 0), stop=(dc == DC - 1))
            nc.vector.tensor_scalar_add(
                dstd[b][:, nj * 512:(nj + 1) * 512], ps[:], bias_sb[:])

        def prologue_p1():
            """b0 k-nj0/q-nj0/v-nj0/k-nj1, d-chunk-major across 4 PSUM
            groups, paced by the half-0 xt piece DMAs."""
            sA = s_ps.tile([128, 1024], f32, name="sA", tag="s")
            vg = po_ps.tile([128, 512], f32, name="vg", tag="po")
            kg1 = po_ps.tile([128, 512], f32, name="kg1", tag="po")
            for dc in range(DC):
                st, sp_ = (dc == 0), (dc == DC - 1)
                p = xt0[dc, 0]
                nc.tensor.matmul(sA[:, 0:512], wk_sb[:, dc * 128:(dc + 1) * 128],
                                 p[:, 0:512], start=st, stop=sp_)
                nc.tensor.matmul(sA[:, 512:1024], wq_sb[:, dc * 128:(dc + 1) * 128],
                                 p[:, 0:512], start=st, stop=sp_)
                nc.tensor.matmul(vg[:], wv_sb[:, dc * 128:(dc + 1) * 128],
                                 p[:, 0:512], start=st, stop=sp_)
                nc.tensor.matmul(kg1[:], wk_sb[:, dc * 128:(dc + 1) * 128],
                                 p[:, 512:1024], start=st, stop=sp_)
            nc.vector.tensor_scalar_add(k2[0][:, 0:512], sA[:, 0:512], bk_sb[:])
            nc.vector.tensor_scalar_add(q2[0][:, 0:512], sA[:, 512:1024], bq_sb[:])
            nc.vector.tensor_scalar_add(vt[0][:, 0:512], vg[:], bv_sb[:])
            nc.vector.tensor_scalar_add(k2[0][:, 512:1024], kg1[:], bk_sb[:])

        def tp_unit(b, nb):
            """Rotate vT[:, nb-chunk] (both heads at once) -> v_sb[n, dh].
            One strided DVE copy scatters the two head halves around the
            shared ones columns."""
            tp = po_ps.tile([128, 128], bf16, name="tp", tag="po")
            nc.tensor.transpose(tp[:], vt[b][:, nb * 128:(nb + 1) * 128],
                                eye_sb[:])
            dst = v_sb[b][:, nb * 192:nb * 192 + 192].rearrange(
                "p (s x) -> p s x", x=64)[:, 0::2, :]
            src = tp[:].rearrange("p (s x) -> p s x", x=64)
            nc.vector.tensor_copy(dst, src)

        def outproj_unit(b, nb, act_copy=False):
            """Partial out rows nb: 2 matmuls -> PSUM->SBUF bf16 copies
            (DVE, or one on ACT in exp-light phases) -> 1 DMA."""
            ob = ob_pool.tile([128, 1024], bf16, name="ob", tag="ob")
            for half in range(2):
                op = po_ps.tile([128, 512], f32, name="op", tag="po")
                nc.tensor.matmul(
                    op[:], sa_sb[b][:, nb * 128:(nb + 1) * 128],
                    wo_sb[:, half * 512:(half + 1) * 512],
                    start=True, stop=True)
                if half == 1 and act_copy:
                    nc.scalar.copy(ob[:, 512:1024], op[:])
                else:
                    nc.vector.tensor_copy(
                        ob[:, half * 512:(half + 1) * 512], op[:])
            nc.sync.dma_start(
                out_d.ap()[b, nb * 128:(nb + 1) * 128, :], ob[:])

        def att_block(b, j, weave=()):
            """One attention j-block: chunk stream with scores-pair /
            merged-exp / delayed-PV pipeline, woven filler, then softmax
            normalize. Output projection of its rows happens as later
            blocks' weave."""
            nch = 4 * (j + 1)
            pv = [pv_ps.tile([128, 512], f32, name=f"pv{h}", tag="pv")
                  for h in range(HL)]
            pts = {}
            weave = list(weave)
            emitted = 0

            def pv_pair(ci):
                t = ci - 4 * j
                lo = 128 * t if t >= 0 else 0
                pt = pts.pop(ci)
                for h in range(HL):
                    nc.tensor.matmul(
                        pv[h][:, lo:512],
                        v_sb[b][:, ci * 192 + 64 * h:ci * 192 + 64 * h + 128],
                        pt[:, 512 * h + lo:512 * h + 512],
                        start=(ci == 0), stop=(ci == nch - 1))

            for ci in range(nch):
                t = ci - 4 * j
                lo = 128 * t if t >= 0 else 0
                # scores, both heads concurrently on PE row-halves
                sp = s_ps.tile([128, 1024], f32, name="s", tag="s")
                for h in range(HL):
                    hp = 64 * h
                    nc.tensor.matmul(
                        sp[:, 512 * h + lo:512 * h + 512],
                        q2[b][hp:hp + 64, ci * 128:(ci + 1) * 128],
                        k2[b][hp:hp + 64, j * 512 + lo:(j + 1) * 512],
                        start=True, stop=True,
                        tile_position=(hp, 0))
                # merged exp over both heads (strided view)
                pt = pt_pool.tile([128, 1024], bf16, name="pt", tag="pt")
                sp3 = sp[:].rearrange("p (h w) -> p h w", h=2)[:, :, lo:512]
                pt3 = pt[:].rearrange("p (h w) -> p h w", h=2)[:, :, lo:512]
                nc.scalar.activation(pt3, sp3, Exp, scale=0.125)
                if t >= 0:
                    for h in range(HL):
                        nc.gpsimd.tensor_tensor(
                            pt[:, 512 * h + lo:512 * h + lo + 128],
                            pt[:, 512 * h + lo:512 * h + lo + 128],
                            m01_sb[:], mybir.AluOpType.mult)
                pts[ci] = pt
                # weave filler before the delayed PV so the PE queue never
                # head-of-line blocks on ACT (one extra unit up front to
                # cover the block-boundary pipeline refill)
                target = min(len(weave), len(weave) * (ci + 1) // nch + 1)
                while emitted < target:
                    weave[emitted]()
                    emitted += 1
                if ci > 1:
                    pv_pair(ci - 2)
            pv_pair(nch - 2)
            pv_pair(nch - 1)

            # normalize: den copies (ACT h0 + DVE h1, in parallel) ->
            # reciprocal (DVE) -> scale (DVE, reads sa rows from PSUM; the
            # PSUM read port allows the h0 downward partition shift, SBUF
            # lanes are fixed). The pv PSUM ring gates the NEXT block's
            # first PV matmul, which is emitted 2 slots late to cover this.
            dens = []
            for h in range(HL):
                den_rows = pv[h][64 - 64 * h:128 - 64 * h, :]
                den_sb = rc_pool.tile([64, 512], f32, name="den", tag="den")
                if h == 0:
                    nc.scalar.copy(den_sb[:], den_rows)
                else:
                    nc.vector.tensor_copy(den_sb[:], den_rows)
                dens.append(den_sb)
            for h in range(HL):
                rc = rc_pool.tile([64, 512], f32, name="rc", tag="rc")
                nc.vector.reciprocal_approx_fast(rc[:], dens[h][:])
                nc.vector.tensor_tensor(
                    sa_sb[b][64 * h:64 * h + 64, j * 512:(j + 1) * 512],
                    pv[h][64 * h:64 * h + 64, :], rc[:],
                    mybir.AluOpType.mult)

        # ================= emission schedule =================
        prologue_p1()
        tp_unit(0, 0)
        tp_unit(0, 1)

        P = proj_group
        T = tp_unit

        def O(b, nb, act=False):
            outproj_unit(b, nb, act_copy=act)

        att_block(0, 0, weave=[
            lambda: P(0, "q", 1), lambda: T(0, 2), lambda: T(0, 3),
            lambda: P(0, "v", 1), lambda: T(0, 4), lambda: T(0, 5),
        ])
        att_block(0, 1, weave=[
            lambda: P(0, "k", 2), lambda: T(0, 6), lambda: T(0, 7),
            lambda: P(0, "q", 2), lambda: P(0, "v", 2),
            lambda: O(0, 0, act=True), lambda: O(0, 1, act=True),
            lambda: T(0, 8),
        ])
        att_block(0, 2, weave=[
            lambda: P(0, "k", 3), lambda: T(0, 9), lambda: T(0, 10),
            lambda: P(0, "q", 3), lambda: P(0, "v", 3),
            lambda: P(1, "k", 0), lambda: P(1, "q", 0), lambda: P(1, "v", 0),
            lambda: T(0, 11), lambda: T(1, 0),
            lambda: O(0, 2, act=True), lambda: O(0, 3, act=True),
        ])
        att_block(1, 0, weave=[
            lambda: P(1, "k", 1), lambda: T(1, 1), lambda: T(1, 2),
            lambda: T(1, 3),
        ])
        att_block(0, 3, weave=[
            lambda: P(1, "q", 1), lambda: T(0, 12), lambda: T(0, 13),
            lambda: T(0, 14), lambda: T(0, 15),
            lambda: P(1, "v", 1), lambda: P(1, "k", 2), lambda: P(1, "q", 2),
            lambda: P(1, "v", 2), lambda: P(1, "k", 3), lambda: P(1, "q", 3),
            lambda: P(1, "v", 3), lambda: T(1, 4), lambda: T(1, 5),
            lambda: O(0, 4, act=True), lambda: O(0, 5, act=True),
            lambda: O(1, 0, act=True), lambda: O(1, 1, act=True),
            lambda: O(1, 2, act=True), lambda: O(1, 3, act=True),
        ])
        att_block(1, 3, weave=[
            lambda: T(1, 6), lambda: T(1, 7), lambda: T(1, 8),
            lambda: T(1, 9), lambda: T(1, 10), lambda: T(1, 11),
            lambda: T(1, 12), lambda: T(1, 13), lambda: T(1, 14),
            lambda: T(1, 15),
            lambda: O(0, 6), lambda: O(0, 7), lambda: O(0, 8),
            lambda: O(0, 9), lambda: O(0, 10), lambda: O(0, 11),
            lambda: O(0, 12), lambda: O(0, 13),
        ])
        att_block(1, 2, weave=[
            lambda: O(0, 14), lambda: O(0, 15),
            lambda: O(1, 12), lambda: O(1, 13), lambda: O(1, 14),
            lambda: O(1, 15),
        ])
        att_block(1, 1, weave=[
            lambda: O(1, 10), lambda: O(1, 11),
        ])
        # pre-tail PE cover: these depend only on norm(1,2), so their
        # matmuls run while the final norm(1,1) DVE chain executes; both
        # copies go to the now-idle ACT so the DVE norm is not delayed
        for nb in (8, 9):
            ob = ob_pool.tile([128, 1024], bf16, name="obp", tag="ob")
            for half in range(2):
                op = po_ps.tile([128, 512], f32, name="opp", tag="po")
                nc.tensor.matmul(
                    op[:], sa_sb[1][:, nb * 128:(nb + 1) * 128],
                    wo_sb[:, half * 512:(half + 1) * 512],
                    start=True, stop=True)
                nc.scalar.copy(ob[:, half * 512:(half + 1) * 512], op[:])
            nc.sync.dma_start(
                out_d.ap()[1, nb * 128:(nb + 1) * 128, :], ob[:])
        # tail: last block's own output projection — pack both halves into
        # one 2-bank s_ps tile (free by now) and split copies ACT/DVE
        for nb in range(4, 8):
            ob = ob_pool.tile([128, 1024], bf16, name="obt", tag="ob")
            sE = s_ps.tile([128, 1024], f32, name="sE", tag="s")
            for half in range(2):
                nc.tensor.matmul(
                    sE[:, half * 512:(half + 1) * 512],
                    sa_sb[1][:, nb * 128:(nb + 1) * 128],
                    wo_sb[:, half * 512:(half + 1) * 512],
                    start=True, stop=True)
            nc.vector.tensor_copy(ob[:, 0:512], sE[:, 0:512])
            nc.scalar.copy(ob[:, 512:1024], sE[:, 512:1024])
            nc.sync.dma_start(
                out_d.ap()[1, nb * 128:(nb + 1) * 128, :], ob[:])

    nc.compile()
    return nc


def _get_nc():
    if "nc" not in _CACHE:
        _CACHE["nc"] = _build()
    return _CACHE["nc"]


def _prep_inputs(x, Wkqv, bkqv, Wo, bo):
    """Host-side shard prep: one input map per core."""
    xt = np.ascontiguousarray(x.transpose(0, 2, 1)).astype(BF16)
    tri = np.triu(np.ones((128, 128), np.float32)).astype(BF16)  # m' <= n''
    eye128 = np.eye(128, dtype=np.float32).astype(BF16)
    in_maps = []
    for c in range(NCORES):
        h0, h1 = HL * c, HL * c + 1
        def shuf(w):
            # [D, 128] -> [128, DC*128]: partition = within-chunk row
            return np.ascontiguousarray(
                w.reshape(DC, 128, 128).transpose(1, 0, 2).reshape(128, DC * 128))

        wk2 = shuf(np.concatenate([Wkqv[h0, :, 0:64], Wkqv[h1, :, 0:64]], axis=1))
        wq2 = shuf(np.concatenate([Wkqv[h0, :, 64:128], Wkqv[h1, :, 64:128]], axis=1))
        wv2 = shuf(np.concatenate([Wkqv[h0, :, 128:192], Wkqv[h1, :, 128:192]], axis=1))
        bk2 = np.concatenate([bkqv[h0, 0:64], bkqv[h1, 0:64]])[:, None]
        bq2 = np.concatenate([bkqv[h0, 64:128], bkqv[h1, 64:128]])[:, None]
        bv2 = np.concatenate([bkqv[h0, 128:192], bkqv[h1, 128:192]])[:, None]
        in_maps.append({
            "xt": xt,
            "wk2": wk2.astype(BF16),
            "wq2": wq2.astype(BF16),
            "wv2": wv2.astype(BF16),
            "wo2": Wo[128 * c:128 * (c + 1), :].astype(BF16),
            "bk2": np.ascontiguousarray(bk2, np.float32),
            "bq2": np.ascontiguousarray(bq2, np.float32),
            "bv2": np.ascontiguousarray(bv2, np.float32),
            "eye128": eye128,
            "m01": tri,
        })
    return in_maps


def kernel(x, Wkqv, bkqv, Wo, bo):
    from concourse import bass_utils

    nc = _get_nc()
    in_maps = _prep_inputs(np.asarray(x), np.asarray(Wkqv), np.asarray(bkqv),
                           np.asarray(Wo), np.asarray(bo))
    res = bass_utils.run_bass_kernel_spmd(nc, in_maps, core_ids=list(range(NCORES)))
    acc = np.zeros((B, N, D), np.float32)
    for c in range(NCORES):
        acc += np.asarray(res.results[c]["out"], np.float32)
    acc += np.asarray(bo)[None, None, :]
    return acc
